# revision 30
# baseline (speedup 1.0000x reference)
"""GAT (3-layer graph attention network) on 8 TRN2 NeuronCores.

Strategy: destination-sharded message passing. Nodes are partitioned across
the 8 cores by destination. Each layer:
  1. node-parallel projection (x @ W_ext, W_ext = [W | W@as | W@ad]) on the
     core's own nodes, building a 256-byte gather-table row per node:
     h as fp8e4 (layers 1-2) or bf16 (layer 3) in bytes 0:128, al_s as
     bf16 at byte 128 (fp8 h keeps the row within the 256-byte dma_gather
     minimum and halves both gather and AllGather traffic vs bf16),
  2. AllGather of the per-core table shards into a full replicated table,
  3. edge-parallel phase, software-pipelined across 128-node blocks
     (gathers/score prep at block nb, alpha-scaling + aggregation matmuls
     at nb-1, softmax-normalize/LayerNorm/ELU post at nb-2, with the
     per-block ald broadcast matmuls precomputed for the whole layer so
     the TensorEngine churns through them under the AllGather): per-edge
     source rows are fetched with dma_gather (256-byte rows, 256 indices
     per call), attention scores exp(LeakyReLU(al_s[src]+al_d[dst])) are
     computed on-chip, and the weighted sum is reduced with static
     "slot -> node" indicator matmuls on the TensorEngine (PSUM
     accumulate). A precise bf16 copy of the local h stays in SBUF for
     the self-loop/post phase, so only remote messages see fp8 rounding.
Edges of each destination node are padded into 4-slot rows; nodes are
grouped by (ceil(nA/4), ceil(nB/4)) degree classes (A/B = source table
half, needed because dma_gather indices are int16) so that all indicator
matrices are static and shared across cores.
"""
import os
import sys
import types
import numpy as np

sys.path.insert(0, "/opt/trn_rl_repo")

import ml_dtypes

D = 4            # slots per row
SEG = 32         # nodes per segment (psum write window)
MAX_CALL_BLOCKS = 2   # indices per dma_gather call = 128*this
NCORES = 8
SINGLE_PACKET = True

bf16_np = ml_dtypes.bfloat16
SENT_ALS = -60000.0


# ----------------------------------------------------------------------
# host-side graph plan
# ----------------------------------------------------------------------

class Plan:
    pass


def build_plan(edge_index, N):
    """Build the shared schedule + per-core index streams."""
    src = np.asarray(edge_index[0], dtype=np.int64)
    dst = np.asarray(edge_index[1], dtype=np.int64)
    # self-loops (PyG default) are handled analytically in the post phase,
    # not as graph edges -- keeps the A/B degree classes core-symmetric.

    NPC = N // NCORES
    owner_half = (src // NPC) < (NCORES // 2)   # True -> A half

    # per-node source lists, split by half
    core_of = dst // NPC
    # class of each node
    a_cnt = np.bincount(dst[owner_half], minlength=N)
    b_cnt = np.bincount(dst[~owner_half], minlength=N)
    kA = -(-a_cnt // D)
    kB = -(-b_cnt // D)

    # collect per-core per-class node lists
    # class key -> per-core list of node ids
    pair_of = kA.astype(np.int64) * 64 + kB.astype(np.int64)
    pairs = {}
    for c in range(NCORES):
        lo, hi = c * NPC, (c + 1) * NPC
        pn = pair_of[lo:hi]
        uniq, counts = np.unique(pn, return_counts=True)
        for u, cnt in zip(uniq, counts):
            pairs.setdefault(int(u), [0] * NCORES)[c] = int(cnt)

    # merge rare classes into a dominating class
    keys = sorted(pairs.keys(), key=lambda u: (-(u // 64 + u % 64), u))
    kept = []
    merge_to = {}
    for u in keys:
        ka, kb = u // 64, u % 64
        mx = max(pairs[u])
        if mx >= 32 or not any(
                (q // 64) >= ka and (q % 64) >= kb for q in kept):
            kept.append(u)
        else:
            cands = [q for q in kept if (q // 64) >= ka and (q % 64) >= kb]
            best = min(cands, key=lambda q: (q // 64 - ka) + (q % 64 - kb))
            merge_to[u] = best
    # final class of every node
    node_cls = pair_of.copy()
    for u, q in merge_to.items():
        node_cls[node_cls == u] = q
    kept = sorted(kept, key=lambda u: (u // 64 + u % 64, u // 64))

    # shared group sizes (32-aligned max across cores)
    cls_size = {}
    for u in kept:
        mx = 0
        for c in range(NCORES):
            lo, hi = c * NPC, (c + 1) * NPC
            mx = max(mx, int((node_cls[lo:hi] == u).sum()))
        cls_size[u] = -(-mx // SEG) * SEG

    node_pad = sum(cls_size.values())
    node_pad_r = -(-node_pad // 128) * 128
    tail_fill = node_pad_r - node_pad      # zero-slot fillers
    NODE_PAD = node_pad_r
    SR = NODE_PAD + 1                      # shard rows (+1 sentinel)
    assert 4 * SR <= 32768, f"table half too big: {4 * SR}"
    NBLK = NODE_PAD // 128

    # per-core node permutation (grouped order); -1 = filler
    perm = np.full((NCORES, NODE_PAD), -1, dtype=np.int64)
    for c in range(NCORES):
        lo, hi = c * NPC, (c + 1) * NPC
        pos = 0
        for u in kept:
            ids = np.nonzero(node_cls[lo:hi] == u)[0] + lo
            perm[c, pos:pos + len(ids)] = ids
            pos += cls_size[u]
        assert pos == node_pad

    # table row of each original node
    row_of = np.full(N, -1, dtype=np.int64)
    for c in range(NCORES):
        valid = perm[c] >= 0
        row_of[perm[c][valid]] = c * SR + np.nonzero(valid)[0]
    assert (row_of >= 0).all()

    # per-node split source row lists (table rows; A rows < 4*SR)
    HALF = 4 * SR
    src_rows = row_of[src]
    order = np.argsort(dst, kind="stable")
    s_sorted = src_rows[order]
    h_sorted = owner_half[order]
    starts = np.searchsorted(dst[order], np.arange(N))
    ends = np.searchsorted(dst[order], np.arange(N) + 1)

    # segment schedule (shared): per 128-node psum block, 4 segments
    seg_cls = []     # class pair per 32-node segment
    for u in kept:
        seg_cls += [(u // 64, u % 64)] * (cls_size[u] // SEG)
    seg_cls += [(0, 0)] * (tail_fill // SEG)
    assert len(seg_cls) == NODE_PAD // SEG

    # slot-block & call schedule per psum block
    blocks = []
    total_cols = 0
    for nb in range(NBLK):
        segs = seg_cls[4 * nb:4 * nb + 4]
        sblocks = []
        for half_i, side in enumerate("AB"):
            for si, (ka, kb) in enumerate(segs):
                k = ka if side == "A" else kb
                for j in range(k):
                    sblocks.append((side, k, j, si))
        calls = []
        i = 0
        while i < len(sblocks):
            side = sblocks[i][0]
            n = 1
            while (n < MAX_CALL_BLOCKS and i + n < len(sblocks)
                   and sblocks[i + n][0] == side):
                n += 1
            calls.append((i, n, side, total_cols))
            total_cols += n * 8     # 128 idx = 8 wrapped cols
            i += n
        blocks.append({"segs": segs, "sblocks": sblocks, "calls": calls,
                       "J": len(sblocks)})
    JMAX = max(b["J"] for b in blocks) if blocks else 0

    # (k, j, si) pattern table for B'/E matrices (full 128-partition lhsT;
    # windowed tile_position is avoided: PE quadrant 3 is unusable)
    pat_ids = {}
    for b in blocks:
        for (_, k, j, si) in b["sblocks"]:
            pat_ids.setdefault((k, j, si), len(pat_ids))
    NPAT = len(pat_ids)

    # static matrices
    bmat = np.zeros((128, NPAT * 128), dtype=bf16_np)   # lhsT [slots, nodes]
    emat = np.zeros((128, NPAT * 128), dtype=bf16_np)   # lhsT [nodes, slots]
    for (k, j, si), pid in pat_ids.items():
        a = 128 * j + np.arange(128)
        nl = si * SEG + a // (D * k)
        bmat[np.arange(128), pid * 128 + nl] = 1.0
        emat[nl, pid * 128 + np.arange(128)] = 1.0

    # per-core idx streams
    idx_stream = np.full((NCORES, 128, total_cols), SR - 1, dtype=np.int16)
    # padded per-node src arrays per core (in grouped node order)
    for c in range(NCORES):
        padded = {}   # side -> [NODE_PAD, D*kmax] per-class handled lazily
        # build per grouped-node lists
        listA = [None] * NODE_PAD
        listB = [None] * NODE_PAD
        for pos in range(NODE_PAD):
            g = perm[c, pos]
            if g < 0:
                listA[pos] = listB[pos] = np.empty(0, dtype=np.int64)
                continue
            sl = slice(starts[g], ends[g])
            ss, hh = s_sorted[sl], h_sorted[sl]
            listA[pos] = ss[hh]
            listB[pos] = ss[~hh] - HALF
        for nb in range(NBLK):
            bl = blocks[nb]
            for sbi, (side, k, j, si) in enumerate(bl["sblocks"]):
                seg_base = nb * 128 + si * SEG
                a = 128 * j + np.arange(128)
                nl = a // (D * k)
                sw = a % (D * k)
                vals = np.full(128, SR - 1, dtype=np.int64)
                lst = listA if side == "A" else listB
                for p in range(128):
                    l = lst[seg_base + nl[p]]
                    if sw[p] < len(l):
                        vals[p] = l[sw[p]]
                # locate call & position
                for (i0, ncall, side2, col0) in bl["calls"]:
                    if i0 <= sbi < i0 + ncall:
                        off = sbi - i0
                        break
                # wrapped layout: idx i of call -> [i % 16, i // 16]
                i_in_call = off * 128 + np.arange(128)
                cols = col0 + i_in_call // 16
                rows = i_in_call % 16
                for rep in range(8):
                    idx_stream[c, rep * 16 + rows, cols] = vals.astype(np.int16)

    p = Plan()
    p.N, p.NPC, p.SR, p.HALF = N, NPC, SR, HALF
    p.NODE_PAD, p.NBLK, p.JMAX, p.NPAT = NODE_PAD, NBLK, JMAX, NPAT
    p.total_cols = total_cols
    p.blocks, p.pat_ids = blocks, pat_ids
    p.perm, p.row_of = perm, row_of
    p.bmat, p.emat = bmat, emat
    p.idx_stream = idx_stream
    p.total_slots = sum(b["J"] for b in blocks) * 128
    return p


# ----------------------------------------------------------------------
# numpy emulation of the planned device computation (for validation)
# ----------------------------------------------------------------------

def _np_layer(p, x_perm, W, a_s, a_d, b, g, be, H, Cd, F):
    """x_perm: [NCORES, NODE_PAD, FIN] in grouped order (fillers zero).
    Returns next x_perm [NCORES, NODE_PAD, F]."""
    FIN = x_perm.shape[2]
    as_bd = np.zeros((H * Cd, H), np.float32)
    ad_bd = np.zeros((H * Cd, H), np.float32)
    for h in range(H):
        as_bd[h * Cd:(h + 1) * Cd, h] = a_s[h]
        ad_bd[h * Cd:(h + 1) * Cd, h] = a_d[h]
    Wext = np.concatenate([W, W @ as_bd, W @ ad_bd], axis=1)  # [FIN, F+2H]

    # projection + table
    proj = np.einsum("cnf,fk->cnk", x_perm, Wext)     # [C, NODE_PAD, F+2H]
    table = np.zeros((NCORES * p.SR, F + H), np.float32)
    for c in range(NCORES):
        table[c * p.SR:c * p.SR + p.NODE_PAD] = proj[c, :, :F + H]
        table[c * p.SR + p.SR - 1, F:] = SENT_ALS  # sentinel als
    al_d = proj[:, :, F + H:]                          # [C, NODE_PAD, H]

    xn = np.zeros((NCORES, p.NODE_PAD, F), np.float32)
    for c in range(NCORES):
        agg = np.zeros((p.NODE_PAD, F + H), np.float32)
        for nb in range(p.NBLK):
            bl = p.blocks[nb]
            for sbi, (side, k, j, si) in enumerate(bl["sblocks"]):
                seg_base = nb * 128 + si * SEG
                # recover idx values from the wrapped stream
                for (i0, ncall, _s, col0) in bl["calls"]:
                    if i0 <= sbi < i0 + ncall:
                        off = sbi - i0
                        break
                i_in_call = off * 128 + np.arange(128)
                cols = col0 + i_in_call // 16
                rows = i_in_call % 16
                vals = p.idx_stream[c, rows, cols].astype(np.int64)
                base = 0 if side == "A" else p.HALF
                grow = base + vals
                hs = table[grow, :F]                  # [128, F]
                als = table[grow, F:]                 # [128, H]
                a = 128 * j + np.arange(128)
                nl = a // (D * k)
                ald = al_d[c, seg_base + nl]          # [128, H]
                e = als + ald
                e = np.where(e > 0, e, 0.2 * e)
                ex = np.exp(e)                        # [128, H]
                sc = (hs.reshape(128, H, Cd) * ex[:, :, None]).reshape(128, F)
                contrib = np.concatenate([sc, ex], 1)
                np.add.at(agg, seg_base + nl, contrib)
        # self-loop contribution (local, no gather)
        h_loc = proj[c, :, :F]
        als_loc = proj[c, :, F:F + H]
        ald_loc = al_d[c]
        e_self = als_loc + ald_loc
        e_self = np.where(e_self > 0, e_self, 0.2 * e_self)
        ex_self = np.exp(e_self)                       # [NODE_PAD, H]
        num = agg[:, :F].reshape(-1, H, Cd) + \
            h_loc.reshape(-1, H, Cd) * ex_self[:, :, None]
        s = agg[:, F:] + ex_self
        outp = num / (s + 1e-16)[:, :, None]
        out = outp.reshape(-1, F) + b
        mu = out.mean(-1, keepdims=True)
        v = ((out - mu) ** 2).mean(-1, keepdims=True)
        ln = g * (out - mu) / np.sqrt(v + 1e-5) + be
        xn[c] = np.where(ln > 0, ln, np.exp(np.minimum(ln, 0)) - 1)
        # zero the fillers
        xn[c][p.perm[c] < 0] = 0.0
    return xn


def plan_emulate(p, inputs):
    """Full numpy emulation following the device plan. Returns [1, 2]."""
    N = p.N
    x = np.asarray(inputs["x"], np.float32)
    x_perm = np.zeros((NCORES, p.NODE_PAD, x.shape[1]), np.float32)
    for c in range(NCORES):
        valid = p.perm[c] >= 0
        x_perm[c][valid] = x[p.perm[c][valid]]

    gi = lambda k: np.asarray(inputs[k], np.float32)
    h = _np_layer(p, x_perm, gi("W1"), gi("as1"), gi("ad1"), gi("b1"),
                  gi("g1"), gi("be1"), 4, 32, 128)
    h = _np_layer(p, h, gi("W2"), gi("as2"), gi("ad2"), gi("b2"),
                  gi("g2"), gi("be2"), 4, 32, 128)
    h = _np_layer(p, h, gi("W3"), gi("as3"), gi("ad3"), gi("b3"),
                  gi("g3"), gi("be3"), 1, 64, 64)
    # pooling
    mask = np.zeros((NCORES, p.NODE_PAD), np.float32)
    for c in range(NCORES):
        mask[c][p.perm[c] >= 0] = 1.0
    hm = h * mask[:, :, None]
    x_sum = hm.sum((0, 1))
    x_max = np.where(mask[:, :, None] > 0, h, -1e9).max((0, 1))
    z = np.concatenate([x_sum / N, x_max])[None, :]
    z = np.maximum(z @ gi("cW1") + gi("cb1"), 0)
    return z @ gi("cW2") + gi("cb2")


# ----------------------------------------------------------------------
# bass kernel builder
# ----------------------------------------------------------------------

LAYER_DIMS = [  # (FIN, F, H, C)
    (512, 128, 4, 32),
    (128, 128, 4, 32),
    (128, 64, 1, 64),
]


def _dt():
    import concourse.mybir as mybir
    return mybir


def build_bass(p, NQ=4, stop=None, repeat=1):
    # stop: 'proj' (L1 proj+AG only), 'l1', 'l2' (n layers, no pooling),
    #       'nopool' (3 layers), None (full)

    import concourse.bass as bass
    import concourse.bacc as bacc
    import concourse.mybir as mybir
    import concourse.tile as tile

    bf16 = mybir.dt.bfloat16
    f32 = mybir.dt.float32
    i16 = mybir.dt.int16
    fp8 = mybir.dt.float8e4

    NODE_PAD, NBLK, SR, HALF = p.NODE_PAD, p.NBLK, p.SR, p.HALF
    JMAX, NPAT = p.JMAX, p.NPAT
    COLS = p.total_cols
    assert all(b["J"] > 0 for b in p.blocks)
    cumJ = [0]
    for b in p.blocks:
        cumJ.append(cumJ[-1] + b["J"])
    TOTJ = cumJ[-1]

    nc = bacc.Bacc("TRN2", target_bir_lowering=False, debug=False,
                   num_devices=NCORES, num_swdge_queues=NQ)

    # ---- dram tensors ----
    din = {}
    def inp(name, shape, dt):
        din[name] = nc.dram_tensor(name, shape, dt, kind="ExternalInput")
        return din[name]

    xT_d = inp("xT", [512, NODE_PAD], bf16)
    idx_d = inp("idx", [128, COLS], i16)
    wext_d = [inp("wext1", [512, 136], bf16),
              inp("wext2", [128, 136], bf16),
              inp("wext3", [128, 66], bf16)]
    rep_d = []
    for li, (_, F, _, _) in enumerate(LAYER_DIMS):
        rep_d.append([inp(f"{nm}{li+1}r", [128, F], f32)
                      for nm in ("b", "g", "be")])
    bmat_d = inp("bmat", [128, NPAT * 128], bf16)
    emat_d = inp("emat", [128, NPAT * 128], bf16)
    ident_d = inp("ident", [128, 128], bf16)
    mask01_d = inp("mask01", [128, NBLK], f32)
    maskneg_d = inp("maskneg", [128, NBLK], f32)
    cw1_d = inp("cw1", [128, 128], f32)
    cb1_d = inp("cb1", [128, 1], f32)
    cw2_d = inp("cw2", [128, 2], f32)
    cb2_d = inp("cb2", [2, 1], f32)

    out_d = nc.dram_tensor("out", [1, 2], f32, kind="ExternalOutput")

    # table row = 256 B: h as fp8 (L1/L2, 128 vals) or bf16 (L3, 64 vals)
    # in bytes 0:128; als as bf16 in bytes 128:128+2H.
    ag_in = nc.dram_tensor("ag_in", [SR, 256], fp8)
    tfull = nc.dram_tensor("tfull", [NCORES * SR, 256], fp8,
                           addr_space="Shared")
    zin = nc.dram_tensor("zin", [128, 1], f32)
    zag = nc.dram_tensor("zag", [NCORES * 128, 1], f32, addr_space="Shared")

    rg = [list(range(NCORES))]
    qn = [0]  # rotating gather queue

    with tile.TileContext(nc) as tc:
        with tc.tile_pool(name="persist", bufs=1) as pp, \
             tc.tile_pool(name="work", bufs=3) as wp, \
             tc.tile_pool(name="gpool", bufs=2) as gp, \
             tc.tile_pool(name="pp_proj", bufs=2, space="PSUM") as ps_proj, \
             tc.tile_pool(name="pp_agg", bufs=2, space="PSUM") as ps_agg, \
             tc.tile_pool(name="pp_esc", bufs=2, space="PSUM") as ps_esc, \
             tc.tile_pool(name="pp_tr", bufs=2, space="PSUM") as ps_tr:

            # ---- load constants ----
            idx_sb = pp.tile([128, COLS], i16, tag="idx")
            nc.sync.dma_start(out=idx_sb[:, :], in_=idx_d[:, :])
            bmat_sb = pp.tile([128, NPAT * 128], bf16, tag="bmat")
            nc.sync.dma_start(out=bmat_sb[:, :], in_=bmat_d[:, :])
            emat_sb = pp.tile([128, NPAT * 128], bf16, tag="emat")
            nc.sync.dma_start(out=emat_sb[:, :], in_=emat_d[:, :])
            ident_sb = pp.tile([128, 128], bf16, tag="ident")
            nc.sync.dma_start(out=ident_sb[:, :], in_=ident_d[:, :])
            mask01_sb = pp.tile([128, NBLK], f32, tag="m01")
            nc.sync.dma_start(out=mask01_sb[:, :], in_=mask01_d[:, :])
            maskneg_sb = pp.tile([128, NBLK], f32, tag="mng")
            nc.sync.dma_start(out=maskneg_sb[:, :], in_=maskneg_d[:, :])
            cw1_sb = pp.tile([128, 128], f32, tag="cw1")
            nc.sync.dma_start(out=cw1_sb[:, :], in_=cw1_d[:, :])
            cb1_sb = pp.tile([128, 1], f32, tag="cb1")
            nc.sync.dma_start(out=cb1_sb[:, :], in_=cb1_d[:, :])
            cw2_sb = pp.tile([128, 2], f32, tag="cw2")
            nc.sync.dma_start(out=cw2_sb[:, :], in_=cw2_d[:, :])
            cb2_sb = pp.tile([2, 1], f32, tag="cb2")
            nc.sync.dma_start(out=cb2_sb[:, :], in_=cb2_d[:, :])
            wext_sb = []
            for li, (FIN, F, H, C) in enumerate(LAYER_DIMS):
                ncol = F + 2 * H
                t = pp.tile([128, (FIN // 128) * ncol], bf16, tag=f"wx{li}")
                for kc in range(FIN // 128):
                    nc.sync.dma_start(
                        out=t[:, kc * ncol:(kc + 1) * ncol],
                        in_=wext_d[li][kc * 128:(kc + 1) * 128, :])
                wext_sb.append(t)
            rep_sb = []
            for li, (_, F, _, _) in enumerate(LAYER_DIMS):
                row = []
                for di, d in enumerate(rep_d[li]):
                    t = pp.tile([128, F], f32, tag=f"rep{li}{di}")
                    nc.sync.dma_start(out=t[:, :], in_=d[:, :])
                    row.append(t)
                rep_sb.append(row)
            ident_f = pp.tile([128, 128], f32, tag="idf")
            nc.vector.tensor_copy(out=ident_f[:, :], in_=ident_sb[:, :])
            ones_f = pp.tile([128, 1], f32, tag="ones")
            nc.vector.memset(ones_f[:, :], 1.0)
            sent_sb = pp.tile([1, 256], fp8, tag="sent")
            nc.vector.memset(sent_sb[:, :], 0.0)

            # resident work tensors
            xT_sb = pp.tile([128, NODE_PAD], bf16, tag="xT")
            xn_sb = pp.tile([128, NBLK * 128], bf16, tag="xn")
            al_sb = pp.tile([128, NBLK * 8], bf16, tag="al")
            hloc_sb = pp.tile([128, NBLK * 128], bf16, tag="hloc")
            esc_all = pp.tile([128, 4 * TOTJ], f32, tag="escall")

            for _rep in range(repeat):
              nlayers = {"proj": 1, "proj_noag": 1, "l1": 1, "l2": 2,
                         "l2_noag": 2, "l2_nogather": 2,
                         "e_gather": 1, "e_esc": 1, "e_mult": 1,
                         "e_agg": 1}.get(stop, 3)
              do_edge = stop not in ("proj", "proj_noag")
              do_pool = stop is None
              elevel = {"e_gather": 1, "e_esc": 2, "e_mult": 3,
                        "e_agg": 4}.get(stop, 99)
              for li, (FIN, F, H, C) in enumerate(LAYER_DIMS[:nlayers]):
                  ncol = F + 2 * H
                  h_fp8 = li < 2          # L3 h fits bytes 0:128 as bf16
                  # ---------------- projection + table ----------------
                  sent_bf = sent_sb[:, :].bitcast(bf16)
                  nc.vector.memset(sent_bf[:, 64:64 + H], SENT_ALS)
                  for nb in range(NBLK):
                      psum_p = ps_proj.tile([128, 144], f32, tag="proj",
                                            space="PSUM")
                      nkc = FIN // 128
                      for kc in range(nkc):
                          if li == 0:
                              lhs = wp.tile([128, 128], bf16, tag="xTc")
                              nc.sync.dma_start(
                                  out=lhs[:, :],
                                  in_=xT_d[kc * 128:(kc + 1) * 128,
                                           nb * 128:(nb + 1) * 128])
                          else:
                              lhs = xT_sb[:, nb * 128:(nb + 1) * 128]
                          nc.tensor.matmul(
                              out=psum_p[:, :ncol],
                              lhsT=lhs[:, :] if li == 0 else lhs,
                              rhs=wext_sb[li][:, kc * ncol:(kc + 1) * ncol],
                              start=(kc == 0), stop=(kc == nkc - 1))
                      tab = wp.tile([128, 256], fp8, tag="tab")
                      tab_bf = tab[:, :].bitcast(bf16)
                      if h_fp8:
                          nc.vector.tensor_copy(out=tab[:, :F],
                                                in_=psum_p[:, :F])
                      else:
                          nc.vector.tensor_copy(out=tab_bf[:, :F],
                                                in_=psum_p[:, :F])
                      nc.vector.tensor_copy(out=tab_bf[:, 64:64 + H],
                                            in_=psum_p[:, F:F + H])
                      nc.vector.tensor_copy(
                          out=al_sb[:, nb * 8:nb * 8 + 2 * H],
                          in_=psum_p[:, F:F + 2 * H])
                      nc.vector.tensor_copy(
                          out=hloc_sb[:, nb * 128:nb * 128 + F],
                          in_=psum_p[:, :F])
                      nc.sync.dma_start(
                          out=ag_in[nb * 128:(nb + 1) * 128, :],
                          in_=tab[:, :])
                  nc.sync.dma_start(out=ag_in[SR - 1:SR, :],
                                    in_=sent_sb[:, :])
                  # esc precompute (PE churns through these during the AG)
                  if do_edge and elevel >= 2:
                      for nb in range(NBLK):
                          bl = p.blocks[nb]
                          J = bl["J"]
                          pe_esc = ps_esc.tile([128, 4 * JMAX], f32,
                                               tag="esc", space="PSUM")
                          for sbi, (side, k, j, si) in enumerate(
                                  bl["sblocks"]):
                              pid = p.pat_ids[(k, j, si)]
                              nc.tensor.matmul(
                                  out=pe_esc[:, sbi * H:(sbi + 1) * H],
                                  lhsT=emat_sb[:, pid * 128:(pid + 1) * 128],
                                  rhs=al_sb[:, nb * 8 + H:nb * 8 + 2 * H],
                                  start=True, stop=True)
                          nc.vector.tensor_copy(
                              out=esc_all[:, H * cumJ[nb]:
                                          H * cumJ[nb] + H * J],
                              in_=pe_esc[:, :H * J])
                  if not (stop == "proj_noag"
                          or (stop == "l2_noag" and li == 1)):
                      nc.gpsimd.collective_compute(
                          "AllGather", mybir.AluOpType.bypass,
                          replica_groups=rg,
                          ins=[ag_in.ap().opt()], outs=[tfull.ap().opt()])

                  # ---------------- edge phase (software-pipelined) -------
                  if not do_edge:
                      break
                  tfA = tfull[0:HALF, :]
                  tfB = tfull[HALF:2 * HALF, :]
                  GJ = F + H
                  Gs, exs, aggs = {}, {}, {}

                  def gather_stage(nb):
                      bl = p.blocks[nb]
                      J = bl["J"]
                      G = gp.tile([128, JMAX * 256], fp8, tag="G")
                      Gs[nb] = G
                      for (i0, ncall, side, col0) in bl["calls"]:
                          nc.gpsimd.dma_gather(
                              out_ap=G[:, i0 * 256:(i0 + ncall) * 256]
                              .rearrange("p (c e) -> p c e", e=256),
                              in_ap=tfA if side == "A" else tfB,
                              idxs_ap=idx_sb[:, col0:col0 + ncall * 8],
                              num_idxs=ncall * 128,
                              num_idxs_reg=ncall * 128,
                              elem_size=256, queue_num=qn[0] % NQ,
                              single_packet=SINGLE_PACKET)
                          qn[0] += 1
                      if elevel < 2:
                          return
                      # bridge copy in the writers' dtype (fp8): the dep
                      # tracker intersects ranges in element units, so a
                      # direct bf16 bitcast read of the multi-call gather
                      # output loses all but the first call's dependency.
                      als_t = wp.tile([128, 8 * JMAX], fp8, tag="alst")
                      nc.vector.tensor_copy(
                          out=als_t[:, :8 * J],
                          in_=G[:, :J * 256].rearrange(
                              "p (c e) -> p c e", e=256)[:, :, 128:136])
                      als_bf = als_t[:, :8 * J].bitcast(bf16).rearrange(
                          "p (c e) -> p c e", e=4)
                      e_f = wp.tile([128, 4 * JMAX], f32, tag="ef")
                      nc.vector.tensor_add(
                          out=e_f[:, :H * J].rearrange(
                              "p (c h) -> p c h", h=H),
                          in0=als_bf[:, :, 0:H],
                          in1=esc_all[:, H * cumJ[nb]:H * cumJ[nb] + H * J]
                          .rearrange("p (c h) -> p c h", h=H))
                      lr = wp.tile([128, 4 * JMAX], f32, tag="lr")
                      nc.vector.tensor_scalar(
                          out=lr[:, :H * J], in0=e_f[:, :H * J],
                          scalar1=0.2, scalar2=None,
                          op0=mybir.AluOpType.mult)
                      nc.vector.tensor_max(
                          out=e_f[:, :H * J], in0=e_f[:, :H * J],
                          in1=lr[:, :H * J])
                      ex = wp.tile([128, 4 * JMAX], bf16, tag="ex")
                      nc.scalar.activation(
                          out=ex[:, :H * J], in_=e_f[:, :H * J],
                          func=mybir.ActivationFunctionType.Exp)
                      exs[nb] = ex

                  def mult_stage(nb):
                      bl = p.blocks[nb]
                      J = bl["J"]
                      G, ex = Gs.pop(nb), exs.pop(nb)
                      Gv = G[:, :J * 256].rearrange("p (c e) -> p c e", e=256)
                      exv = ex[:, :H * J].rearrange("p (c h) -> p c h", h=H)
                      Gm = gp.tile([128, JMAX * 132], bf16, tag="Gm")
                      Gmv = Gm[:, :J * GJ].rearrange("p (c e) -> p c e", e=GJ)
                      if h_fp8:
                          h_src = Gv[:, :, 0:F]
                      else:
                          # same-dtype bridge for the bf16 h bitcast (L3)
                          ht = wp.tile([128, 128 * JMAX], fp8, tag="ht")
                          nc.vector.tensor_copy(out=ht[:, :J * 128],
                                                in_=Gv[:, :, 0:128])
                          h_src = ht[:, :J * 128].bitcast(bf16).rearrange(
                              "p (c e) -> p c e", e=64)[:, :, 0:F]
                      nc.vector.tensor_tensor(
                          out=Gmv[:, :, 0:F].rearrange(
                              "p c (h w) -> p c h w", h=H),
                          in0=h_src.rearrange("p c (h w) -> p c h w", h=H),
                          in1=exv.unsqueeze(3).to_broadcast([128, J, H, C]),
                          op=mybir.AluOpType.mult)
                      nc.vector.tensor_copy(out=Gmv[:, :, F:F + H], in_=exv)
                      if elevel < 4:
                          return
                      agg = ps_agg.tile([128, 144], f32, tag="agg",
                                        space="PSUM")
                      aggs[nb] = agg
                      for sbi, (side, k, j, si) in enumerate(bl["sblocks"]):
                          pid = p.pat_ids[(k, j, si)]
                          nc.tensor.matmul(
                              out=agg[:, :F + H],
                              lhsT=bmat_sb[:, pid * 128:(pid + 1) * 128],
                              rhs=Gm[:, sbi * GJ:sbi * GJ + F + H],
                              start=(sbi == 0), stop=(sbi == J - 1),
                              skip_group_check=True)

                  def post_stage(nb):
                      agg = aggs.pop(nb)
                      es = wp.tile([128, 4], f32, tag="es")
                      nc.vector.tensor_add(
                          out=es[:, :H], in0=al_sb[:, nb * 8:nb * 8 + H],
                          in1=al_sb[:, nb * 8 + H:nb * 8 + 2 * H])
                      lr2 = wp.tile([128, 4], f32, tag="lr2")
                      nc.vector.tensor_scalar(
                          out=lr2[:, :H], in0=es[:, :H], scalar1=0.2,
                          scalar2=None, op0=mybir.AluOpType.mult)
                      nc.vector.tensor_max(out=es[:, :H], in0=es[:, :H],
                                           in1=lr2[:, :H])
                      nc.scalar.activation(
                          out=es[:, :H], in_=es[:, :H],
                          func=mybir.ActivationFunctionType.Exp)
                      num = wp.tile([128, 128], f32, tag="num")
                      nc.vector.tensor_tensor(
                          out=num[:, :F].rearrange("p (h w) -> p h w", h=H),
                          in0=hloc_sb[:, nb * 128:nb * 128 + F].rearrange(
                              "p (h w) -> p h w", h=H),
                          in1=es[:, :H].unsqueeze(2).to_broadcast([128, H, C]),
                          op=mybir.AluOpType.mult)
                      nc.vector.tensor_add(out=num[:, :F], in0=num[:, :F],
                                           in1=agg[:, 0:F])
                      den = wp.tile([128, 4], f32, tag="den")
                      nc.vector.tensor_add(out=den[:, :H], in0=agg[:, F:F + H],
                                           in1=es[:, :H])
                      nc.vector.tensor_scalar_add(out=den[:, :H],
                                                  in0=den[:, :H],
                                                  scalar1=1e-16)
                      rec = wp.tile([128, 4], f32, tag="rec")
                      nc.vector.reciprocal(out=rec[:, :H], in_=den[:, :H])
                      xw = wp.tile([128, 128], f32, tag="xw")
                      nc.vector.tensor_tensor(
                          out=xw[:, :F].rearrange("p (h w) -> p h w", h=H),
                          in0=num[:, :F].rearrange("p (h w) -> p h w", h=H),
                          in1=rec[:, :H].unsqueeze(2).to_broadcast([128, H, C]),
                          op=mybir.AluOpType.mult)
                      nc.vector.tensor_add(out=xw[:, :F], in0=xw[:, :F],
                                           in1=rep_sb[li][0][:, :])
                      # layernorm
                      trash = wp.tile([128, 128], f32, tag="trash")
                      mu = wp.tile([128, 1], f32, tag="mu")
                      nc.scalar.activation(
                          out=trash[:, :F], in_=xw[:, :F],
                          func=mybir.ActivationFunctionType.Identity,
                          scale=1.0 / F, accum_out=mu[:, :])
                      cen = wp.tile([128, 128], f32, tag="cen")
                      nc.vector.tensor_scalar(
                          out=cen[:, :F], in0=xw[:, :F], scalar1=mu[:, :],
                          scalar2=None, op0=mybir.AluOpType.subtract)
                      sq = wp.tile([128, 1], f32, tag="sq")
                      nc.scalar.activation(
                          out=trash[:, :F], in_=cen[:, :F],
                          func=mybir.ActivationFunctionType.Square,
                          accum_out=sq[:, :])
                      rstd = wp.tile([128, 1], f32, tag="rstd")
                      nc.vector.tensor_scalar(
                          out=rstd[:, :], in0=sq[:, :], scalar1=1.0 / F,
                          scalar2=1e-5, op0=mybir.AluOpType.mult,
                          op1=mybir.AluOpType.add)
                      nc.scalar.activation(
                          out=rstd[:, :], in_=rstd[:, :],
                          func=mybir.ActivationFunctionType.Sqrt)
                      nc.vector.reciprocal(out=rstd[:, :], in_=rstd[:, :])
                      nc.vector.tensor_scalar(
                          out=cen[:, :F], in0=cen[:, :F], scalar1=rstd[:, :],
                          scalar2=None, op0=mybir.AluOpType.mult)
                      nc.vector.tensor_mul(out=cen[:, :F], in0=cen[:, :F],
                                           in1=rep_sb[li][1][:, :])
                      nc.vector.tensor_add(out=cen[:, :F], in0=cen[:, :F],
                                           in1=rep_sb[li][2][:, :])
                      # ELU
                      t3 = wp.tile([128, 128], f32, tag="t3")
                      nc.vector.tensor_scalar(
                          out=t3[:, :F], in0=cen[:, :F], scalar1=0.0,
                          scalar2=None, op0=mybir.AluOpType.min)
                      nc.scalar.activation(
                          out=t3[:, :F], in_=t3[:, :F],
                          func=mybir.ActivationFunctionType.Exp)
                      nc.vector.tensor_scalar_add(out=t3[:, :F],
                                                  in0=t3[:, :F], scalar1=-1.0)
                      nc.vector.tensor_tensor(
                          out=xn_sb[:, nb * F:(nb + 1) * F],
                          in0=cen[:, :F], in1=t3[:, :F],
                          op=mybir.AluOpType.max)

                  for it in range(NBLK + 2):
                      if it < NBLK:
                          gather_stage(it)
                      if elevel >= 3 and 0 <= it - 1 < NBLK:
                          mult_stage(it - 1)
                      if elevel >= 5 and 0 <= it - 2 < NBLK:
                          post_stage(it - 2)
                  # transpose xn -> xT for next layer
                  if li < nlayers - 1 and elevel >= 5:
                      for nb in range(NBLK):
                          pt = ps_tr.tile([128, 128], bf16, tag="tr",
                                          space="PSUM")
                          nc.tensor.transpose(
                              out=pt[:, :],
                              in_=xn_sb[:, nb * 128:(nb + 1) * 128],
                              identity=ident_sb[:, :])
                          nc.vector.tensor_copy(
                              out=xT_sb[:, nb * 128:(nb + 1) * 128],
                              in_=pt[:, :])

              # ---------------- pooling + classifier ----------------
              if not do_pool:
                  o0 = wp.tile([1, 2], f32, tag="o0")
                  nc.vector.memset(o0[:, :], 0.0)
                  nc.sync.dma_start(out=out_d[:, :], in_=o0[:, :])
              else:
                F3 = 64
                sumacc = pp.tile([128, F3], f32, tag="sumacc")
                nc.vector.memset(sumacc[:, :], 0.0)
                maxacc = pp.tile([128, F3], f32, tag="maxacc")
                nc.vector.memset(maxacc[:, :], -1e9)
                for nb in range(NBLK):
                    blk = xn_sb[:, nb * F3:(nb + 1) * F3]
                    hm = wp.tile([128, F3], f32, tag="hm")
                    nc.vector.tensor_scalar(
                        out=hm[:, :], in0=blk, scalar1=mask01_sb[:, nb:nb + 1],
                        scalar2=None, op0=mybir.AluOpType.mult)
                    nc.vector.tensor_add(out=sumacc[:, :], in0=sumacc[:, :],
                                         in1=hm[:, :])
                    nc.vector.tensor_scalar(
                        out=hm[:, :], in0=blk, scalar1=maskneg_sb[:, nb:nb + 1],
                        scalar2=None, op0=mybir.AluOpType.add)
                    nc.vector.tensor_max(out=maxacc[:, :], in0=maxacc[:, :],
                                         in1=hm[:, :])
                sps = ps_tr.tile([64, 1], f32, tag="tr", space="PSUM")
                nc.tensor.matmul(out=sps[:, :], lhsT=sumacc[:, :],
                                 rhs=ones_f[:, :], start=True, stop=True)
                s1 = wp.tile([64, 1], f32, tag="s1")
                nc.vector.tensor_copy(out=s1[:, :], in_=sps[:, :])
                mps = ps_tr.tile([64, 128], f32, tag="tr", space="PSUM")
                nc.tensor.matmul(out=mps[:, :], lhsT=maxacc[:, :],
                                 rhs=ident_f[:, :], start=True, stop=True,
                                 is_transpose=True)
                m1 = wp.tile([64, 1], f32, tag="m1")
                nc.vector.tensor_reduce(out=m1[:, :], in_=mps[:, :],
                                        axis=mybir.AxisListType.X,
                                        op=mybir.AluOpType.max)
                nc.sync.dma_start(out=zin[0:64, :], in_=s1[:, :])
                nc.sync.dma_start(out=zin[64:128, :], in_=m1[:, :])
                nc.gpsimd.collective_compute(
                    "AllGather", mybir.AluOpType.bypass, replica_groups=rg,
                    ins=[zin.ap().opt()], outs=[zag.ap().opt()])
                zs = wp.tile([128, 8], f32, tag="zs")
                for c in range(NCORES):
                    nc.sync.dma_start(out=zs[:, c:c + 1],
                                      in_=zag[c * 128:(c + 1) * 128, :])
                z2 = wp.tile([128, 1], f32, tag="z2")
                nc.vector.tensor_reduce(out=z2[0:64, :], in_=zs[0:64, :],
                                        axis=mybir.AxisListType.X,
                                        op=mybir.AluOpType.add)
                nc.vector.tensor_reduce(out=z2[64:128, :], in_=zs[64:128, :],
                                        axis=mybir.AxisListType.X,
                                        op=mybir.AluOpType.max)
                f1 = ps_tr.tile([128, 1], f32, tag="tr", space="PSUM")
                nc.tensor.matmul(out=f1[:, :], lhsT=cw1_sb[:, :], rhs=z2[:, :],
                                 start=True, stop=True)
                r1 = wp.tile([128, 1], f32, tag="r1")
                nc.scalar.activation(out=r1[:, :], in_=f1[:, :],
                                     func=mybir.ActivationFunctionType.Relu,
                                     bias=cb1_sb[:, :])
                f2 = ps_tr.tile([2, 1], f32, tag="tr", space="PSUM")
                nc.tensor.matmul(out=f2[:, :], lhsT=cw2_sb[:, :], rhs=r1[:, :],
                                 start=True, stop=True)
                o = wp.tile([2, 1], f32, tag="o")
                nc.vector.tensor_add(out=o[:, :], in0=f2[:, :], in1=cb2_sb[:, :])
                nc.sync.dma_start(out=out_d[:, :], in_=o[:, :])
    nc.compile()
    return nc


def build_in_maps(p, inputs):
    """Per-core input dicts."""
    x = np.asarray(inputs["x"], np.float32)
    FIN = x.shape[1]
    in_maps = []
    gi = lambda k: np.asarray(inputs[k], np.float32)

    wext, reps = [], []
    for li, (fin, F, H, C) in enumerate(LAYER_DIMS):
        W = gi(f"W{li+1}")
        a_s, a_d = gi(f"as{li+1}"), gi(f"ad{li+1}")
        as_bd = np.zeros((H * C, H), np.float32)
        ad_bd = np.zeros((H * C, H), np.float32)
        for h in range(H):
            as_bd[h * C:(h + 1) * C, h] = a_s[h]
            ad_bd[h * C:(h + 1) * C, h] = a_d[h]
        wext.append(np.concatenate(
            [W, W @ as_bd, W @ ad_bd], axis=1).astype(bf16_np))
        reps.append([np.tile(gi(f"{nm}{li+1}")[None, :], (128, 1))
                     .astype(np.float32) for nm in ("b", "g", "be")])

    ident = np.eye(128, dtype=bf16_np)
    cw1 = gi("cW1").copy()
    cw1[:64, :] /= p.N          # fold mean divisor into cW1 rows
    cb1 = gi("cb1")[:, None].astype(np.float32)
    cw2 = gi("cW2").astype(np.float32)
    cb2 = gi("cb2")[:, None].astype(np.float32)

    for c in range(NCORES):
        m = {}
        xp = np.zeros((p.NODE_PAD, FIN), np.float32)
        valid = p.perm[c] >= 0
        xp[valid] = x[p.perm[c][valid]]
        m["xT"] = np.ascontiguousarray(xp.T).astype(bf16_np)
        m["idx"] = p.idx_stream[c]
        for li in range(3):
            m[f"wext{li+1}"] = wext[li]
            for di, nm in enumerate(("b", "g", "be")):
                m[f"{nm}{li+1}r"] = reps[li][di]
        m["bmat"] = p.bmat
        m["emat"] = p.emat
        m["ident"] = ident
        mk = valid.astype(np.float32)
        m["mask01"] = mk.reshape(p.NBLK, 128).T.copy()
        m["maskneg"] = ((1.0 - mk) * -1e9).reshape(p.NBLK, 128).T.copy()
        m["cw1"] = cw1.astype(np.float32)
        m["cb1"] = cb1
        m["cw2"] = cw2
        m["cb2"] = cb2
        in_maps.append(m)
    return in_maps


_CACHE = {}


def kernel(**inputs):
    from concourse.bass_utils import run_bass_kernel_spmd
    os.environ.setdefault("NEURON_RT_RESET_CORES", "1")
    key = "k"
    if key not in _CACHE:
        p = build_plan(np.asarray(inputs["edge_index"]),
                       int(np.asarray(inputs["x"]).shape[0]))
        nc = build_bass(p)
        _CACHE[key] = (p, nc)
    p, nc = _CACHE[key]
    in_maps = build_in_maps(p, inputs)
    res = run_bass_kernel_spmd(nc, in_maps, core_ids=list(range(NCORES)))
    return res.results[0]["out"].astype(np.float32)



# revision 33
# speedup vs baseline: 1.0403x; 1.0403x over previous
"""GAT (3-layer graph attention network) on 8 TRN2 NeuronCores.

Strategy: destination-sharded message passing. Nodes are partitioned across
the 8 cores by destination. Each layer:
  1. node-parallel projection (x @ W_ext, W_ext = [W | W@as | W@ad]) on the
     core's own nodes, building a 256-byte gather-table row per node:
     h as fp8e4 (layers 1-2) or bf16 (layer 3) in bytes 0:128, al_s as
     bf16 at byte 128 (fp8 h keeps the row within the 256-byte dma_gather
     minimum and halves both gather and AllGather traffic vs bf16),
  2. AllGather of the per-core table shards into a full replicated table,
  3. edge-parallel phase, software-pipelined across 128-node blocks
     (gathers/score prep at block nb, alpha-scaling + aggregation matmuls
     at nb-1, softmax-normalize/LayerNorm/ELU post at nb-2, with the
     per-block ald broadcast matmuls precomputed for the whole layer so
     the TensorEngine churns through them under the AllGather): per-edge
     source rows are fetched with dma_gather (256-byte rows, 256 indices
     per call), attention scores exp(LeakyReLU(al_s[src]+al_d[dst])) are
     computed on-chip, and the weighted sum is reduced with static
     "slot -> node" indicator matmuls on the TensorEngine (PSUM
     accumulate). A precise bf16 copy of the local h stays in SBUF for
     the self-loop/post phase, so only remote messages see fp8 rounding.
Edges of each destination node are padded into 4-slot rows; nodes are
grouped by (ceil(nA/4), ceil(nB/4)) degree classes (A/B = source table
half, needed because dma_gather indices are int16) so that all indicator
matrices are static and shared across cores.
"""
import os
import sys
import types
import numpy as np

sys.path.insert(0, "/opt/trn_rl_repo")

import ml_dtypes

D = 4            # slots per row
SEG = 32         # nodes per segment (psum write window)
MAX_CALL_BLOCKS = 2   # indices per dma_gather call = 128*this
NCORES = 8
SINGLE_PACKET = True

bf16_np = ml_dtypes.bfloat16
SENT_ALS = -60000.0


# ----------------------------------------------------------------------
# host-side graph plan
# ----------------------------------------------------------------------

class Plan:
    pass


def build_plan(edge_index, N):
    """Build the shared schedule + per-core index streams."""
    src = np.asarray(edge_index[0], dtype=np.int64)
    dst = np.asarray(edge_index[1], dtype=np.int64)
    # self-loops (PyG default) are handled analytically in the post phase,
    # not as graph edges -- keeps the A/B degree classes core-symmetric.

    NPC = N // NCORES
    owner_half = (src // NPC) < (NCORES // 2)   # True -> A half

    # per-node source lists, split by half
    core_of = dst // NPC
    # class of each node
    a_cnt = np.bincount(dst[owner_half], minlength=N)
    b_cnt = np.bincount(dst[~owner_half], minlength=N)
    kA = -(-a_cnt // D)
    kB = -(-b_cnt // D)

    # collect per-core per-class node lists
    # class key -> per-core list of node ids
    pair_of = kA.astype(np.int64) * 64 + kB.astype(np.int64)
    pairs = {}
    for c in range(NCORES):
        lo, hi = c * NPC, (c + 1) * NPC
        pn = pair_of[lo:hi]
        uniq, counts = np.unique(pn, return_counts=True)
        for u, cnt in zip(uniq, counts):
            pairs.setdefault(int(u), [0] * NCORES)[c] = int(cnt)

    # merge rare classes into a dominating class
    keys = sorted(pairs.keys(), key=lambda u: (-(u // 64 + u % 64), u))
    kept = []
    merge_to = {}
    for u in keys:
        ka, kb = u // 64, u % 64
        mx = max(pairs[u])
        if mx >= 32 or not any(
                (q // 64) >= ka and (q % 64) >= kb for q in kept):
            kept.append(u)
        else:
            cands = [q for q in kept if (q // 64) >= ka and (q % 64) >= kb]
            best = min(cands, key=lambda q: (q // 64 - ka) + (q % 64 - kb))
            merge_to[u] = best
    # final class of every node
    node_cls = pair_of.copy()
    for u, q in merge_to.items():
        node_cls[node_cls == u] = q
    kept = sorted(kept, key=lambda u: (u // 64 + u % 64, u // 64))

    # shared group sizes (32-aligned max across cores)
    cls_size = {}
    for u in kept:
        mx = 0
        for c in range(NCORES):
            lo, hi = c * NPC, (c + 1) * NPC
            mx = max(mx, int((node_cls[lo:hi] == u).sum()))
        cls_size[u] = -(-mx // SEG) * SEG

    node_pad = sum(cls_size.values())
    node_pad_r = -(-node_pad // 128) * 128
    tail_fill = node_pad_r - node_pad      # zero-slot fillers
    NODE_PAD = node_pad_r
    SR = NODE_PAD + 1                      # shard rows (+1 sentinel)
    assert 4 * SR <= 32768, f"table half too big: {4 * SR}"
    NBLK = NODE_PAD // 128

    # per-core node permutation (grouped order); -1 = filler
    perm = np.full((NCORES, NODE_PAD), -1, dtype=np.int64)
    for c in range(NCORES):
        lo, hi = c * NPC, (c + 1) * NPC
        pos = 0
        for u in kept:
            ids = np.nonzero(node_cls[lo:hi] == u)[0] + lo
            perm[c, pos:pos + len(ids)] = ids
            pos += cls_size[u]
        assert pos == node_pad

    # table row of each original node
    row_of = np.full(N, -1, dtype=np.int64)
    for c in range(NCORES):
        valid = perm[c] >= 0
        row_of[perm[c][valid]] = c * SR + np.nonzero(valid)[0]
    assert (row_of >= 0).all()

    # per-node split source row lists (table rows; A rows < 4*SR)
    HALF = 4 * SR
    src_rows = row_of[src]
    order = np.argsort(dst, kind="stable")
    s_sorted = src_rows[order]
    h_sorted = owner_half[order]
    starts = np.searchsorted(dst[order], np.arange(N))
    ends = np.searchsorted(dst[order], np.arange(N) + 1)

    # segment schedule (shared): per 128-node psum block, 4 segments
    seg_cls = []     # class pair per 32-node segment
    for u in kept:
        seg_cls += [(u // 64, u % 64)] * (cls_size[u] // SEG)
    seg_cls += [(0, 0)] * (tail_fill // SEG)
    assert len(seg_cls) == NODE_PAD // SEG

    # slot-block & call schedule per psum block
    blocks = []
    total_cols = 0
    for nb in range(NBLK):
        segs = seg_cls[4 * nb:4 * nb + 4]
        sblocks = []
        for half_i, side in enumerate("AB"):
            for si, (ka, kb) in enumerate(segs):
                k = ka if side == "A" else kb
                for j in range(k):
                    sblocks.append((side, k, j, si))
        calls = []
        i = 0
        while i < len(sblocks):
            side = sblocks[i][0]
            n = 1
            while (n < MAX_CALL_BLOCKS and i + n < len(sblocks)
                   and sblocks[i + n][0] == side):
                n += 1
            calls.append((i, n, side, total_cols))
            total_cols += n * 8     # 128 idx = 8 wrapped cols
            i += n
        blocks.append({"segs": segs, "sblocks": sblocks, "calls": calls,
                       "J": len(sblocks)})
    JMAX = max(b["J"] for b in blocks) if blocks else 0

    # (k, j, si) pattern table for B'/E matrices (full 128-partition lhsT;
    # windowed tile_position is avoided: PE quadrant 3 is unusable)
    pat_ids = {}
    for b in blocks:
        for (_, k, j, si) in b["sblocks"]:
            pat_ids.setdefault((k, j, si), len(pat_ids))
    NPAT = len(pat_ids)

    # static matrices
    bmat = np.zeros((128, NPAT * 128), dtype=bf16_np)   # lhsT [slots, nodes]
    emat = np.zeros((128, NPAT * 128), dtype=bf16_np)   # lhsT [nodes, slots]
    for (k, j, si), pid in pat_ids.items():
        a = 128 * j + np.arange(128)
        nl = si * SEG + a // (D * k)
        bmat[np.arange(128), pid * 128 + nl] = 1.0
        emat[nl, pid * 128 + np.arange(128)] = 1.0

    # per-core idx streams
    idx_stream = np.full((NCORES, 128, total_cols), SR - 1, dtype=np.int16)
    # padded per-node src arrays per core (in grouped node order)
    for c in range(NCORES):
        padded = {}   # side -> [NODE_PAD, D*kmax] per-class handled lazily
        # build per grouped-node lists
        listA = [None] * NODE_PAD
        listB = [None] * NODE_PAD
        for pos in range(NODE_PAD):
            g = perm[c, pos]
            if g < 0:
                listA[pos] = listB[pos] = np.empty(0, dtype=np.int64)
                continue
            sl = slice(starts[g], ends[g])
            ss, hh = s_sorted[sl], h_sorted[sl]
            listA[pos] = ss[hh]
            listB[pos] = ss[~hh] - HALF
        for nb in range(NBLK):
            bl = blocks[nb]
            for sbi, (side, k, j, si) in enumerate(bl["sblocks"]):
                seg_base = nb * 128 + si * SEG
                a = 128 * j + np.arange(128)
                nl = a // (D * k)
                sw = a % (D * k)
                vals = np.full(128, SR - 1, dtype=np.int64)
                lst = listA if side == "A" else listB
                for p in range(128):
                    l = lst[seg_base + nl[p]]
                    if sw[p] < len(l):
                        vals[p] = l[sw[p]]
                # locate call & position
                for (i0, ncall, side2, col0) in bl["calls"]:
                    if i0 <= sbi < i0 + ncall:
                        off = sbi - i0
                        break
                # wrapped layout: idx i of call -> [i % 16, i // 16]
                i_in_call = off * 128 + np.arange(128)
                cols = col0 + i_in_call // 16
                rows = i_in_call % 16
                for rep in range(8):
                    idx_stream[c, rep * 16 + rows, cols] = vals.astype(np.int16)

    p = Plan()
    p.N, p.NPC, p.SR, p.HALF = N, NPC, SR, HALF
    p.NODE_PAD, p.NBLK, p.JMAX, p.NPAT = NODE_PAD, NBLK, JMAX, NPAT
    p.total_cols = total_cols
    p.blocks, p.pat_ids = blocks, pat_ids
    p.perm, p.row_of = perm, row_of
    p.bmat, p.emat = bmat, emat
    p.idx_stream = idx_stream
    p.total_slots = sum(b["J"] for b in blocks) * 128
    return p


# ----------------------------------------------------------------------
# numpy emulation of the planned device computation (for validation)
# ----------------------------------------------------------------------

def _np_layer(p, x_perm, W, a_s, a_d, b, g, be, H, Cd, F):
    """x_perm: [NCORES, NODE_PAD, FIN] in grouped order (fillers zero).
    Returns next x_perm [NCORES, NODE_PAD, F]."""
    FIN = x_perm.shape[2]
    as_bd = np.zeros((H * Cd, H), np.float32)
    ad_bd = np.zeros((H * Cd, H), np.float32)
    for h in range(H):
        as_bd[h * Cd:(h + 1) * Cd, h] = a_s[h]
        ad_bd[h * Cd:(h + 1) * Cd, h] = a_d[h]
    Wext = np.concatenate([W, W @ as_bd, W @ ad_bd], axis=1)  # [FIN, F+2H]

    # projection + table
    proj = np.einsum("cnf,fk->cnk", x_perm, Wext)     # [C, NODE_PAD, F+2H]
    table = np.zeros((NCORES * p.SR, F + H), np.float32)
    for c in range(NCORES):
        table[c * p.SR:c * p.SR + p.NODE_PAD] = proj[c, :, :F + H]
        table[c * p.SR + p.SR - 1, F:] = SENT_ALS  # sentinel als
    al_d = proj[:, :, F + H:]                          # [C, NODE_PAD, H]

    xn = np.zeros((NCORES, p.NODE_PAD, F), np.float32)
    for c in range(NCORES):
        agg = np.zeros((p.NODE_PAD, F + H), np.float32)
        for nb in range(p.NBLK):
            bl = p.blocks[nb]
            for sbi, (side, k, j, si) in enumerate(bl["sblocks"]):
                seg_base = nb * 128 + si * SEG
                # recover idx values from the wrapped stream
                for (i0, ncall, _s, col0) in bl["calls"]:
                    if i0 <= sbi < i0 + ncall:
                        off = sbi - i0
                        break
                i_in_call = off * 128 + np.arange(128)
                cols = col0 + i_in_call // 16
                rows = i_in_call % 16
                vals = p.idx_stream[c, rows, cols].astype(np.int64)
                base = 0 if side == "A" else p.HALF
                grow = base + vals
                hs = table[grow, :F]                  # [128, F]
                als = table[grow, F:]                 # [128, H]
                a = 128 * j + np.arange(128)
                nl = a // (D * k)
                ald = al_d[c, seg_base + nl]          # [128, H]
                e = als + ald
                e = np.where(e > 0, e, 0.2 * e)
                ex = np.exp(e)                        # [128, H]
                sc = (hs.reshape(128, H, Cd) * ex[:, :, None]).reshape(128, F)
                contrib = np.concatenate([sc, ex], 1)
                np.add.at(agg, seg_base + nl, contrib)
        # self-loop contribution (local, no gather)
        h_loc = proj[c, :, :F]
        als_loc = proj[c, :, F:F + H]
        ald_loc = al_d[c]
        e_self = als_loc + ald_loc
        e_self = np.where(e_self > 0, e_self, 0.2 * e_self)
        ex_self = np.exp(e_self)                       # [NODE_PAD, H]
        num = agg[:, :F].reshape(-1, H, Cd) + \
            h_loc.reshape(-1, H, Cd) * ex_self[:, :, None]
        s = agg[:, F:] + ex_self
        outp = num / (s + 1e-16)[:, :, None]
        out = outp.reshape(-1, F) + b
        mu = out.mean(-1, keepdims=True)
        v = ((out - mu) ** 2).mean(-1, keepdims=True)
        ln = g * (out - mu) / np.sqrt(v + 1e-5) + be
        xn[c] = np.where(ln > 0, ln, np.exp(np.minimum(ln, 0)) - 1)
        # zero the fillers
        xn[c][p.perm[c] < 0] = 0.0
    return xn


def plan_emulate(p, inputs):
    """Full numpy emulation following the device plan. Returns [1, 2]."""
    N = p.N
    x = np.asarray(inputs["x"], np.float32)
    x_perm = np.zeros((NCORES, p.NODE_PAD, x.shape[1]), np.float32)
    for c in range(NCORES):
        valid = p.perm[c] >= 0
        x_perm[c][valid] = x[p.perm[c][valid]]

    gi = lambda k: np.asarray(inputs[k], np.float32)
    h = _np_layer(p, x_perm, gi("W1"), gi("as1"), gi("ad1"), gi("b1"),
                  gi("g1"), gi("be1"), 4, 32, 128)
    h = _np_layer(p, h, gi("W2"), gi("as2"), gi("ad2"), gi("b2"),
                  gi("g2"), gi("be2"), 4, 32, 128)
    h = _np_layer(p, h, gi("W3"), gi("as3"), gi("ad3"), gi("b3"),
                  gi("g3"), gi("be3"), 1, 64, 64)
    # pooling
    mask = np.zeros((NCORES, p.NODE_PAD), np.float32)
    for c in range(NCORES):
        mask[c][p.perm[c] >= 0] = 1.0
    hm = h * mask[:, :, None]
    x_sum = hm.sum((0, 1))
    x_max = np.where(mask[:, :, None] > 0, h, -1e9).max((0, 1))
    z = np.concatenate([x_sum / N, x_max])[None, :]
    z = np.maximum(z @ gi("cW1") + gi("cb1"), 0)
    return z @ gi("cW2") + gi("cb2")


# ----------------------------------------------------------------------
# bass kernel builder
# ----------------------------------------------------------------------

LAYER_DIMS = [  # (FIN, F, H, C)
    (512, 128, 4, 32),
    (128, 128, 4, 32),
    (128, 64, 1, 64),
]


def _dt():
    import concourse.mybir as mybir
    return mybir


def build_bass(p, NQ=4, stop=None, repeat=1):
    # stop: 'proj' (L1 proj+AG only), 'l1', 'l2' (n layers, no pooling),
    #       'nopool' (3 layers), None (full)

    import concourse.bass as bass
    import concourse.bacc as bacc
    import concourse.mybir as mybir
    import concourse.tile as tile

    bf16 = mybir.dt.bfloat16
    f32 = mybir.dt.float32
    i16 = mybir.dt.int16
    fp8 = mybir.dt.float8e4

    NODE_PAD, NBLK, SR, HALF = p.NODE_PAD, p.NBLK, p.SR, p.HALF
    JMAX, NPAT = p.JMAX, p.NPAT
    COLS = p.total_cols
    assert all(b["J"] > 0 for b in p.blocks)
    cumJ = [0]
    for b in p.blocks:
        cumJ.append(cumJ[-1] + b["J"])
    TOTJ = cumJ[-1]

    nc = bacc.Bacc("TRN2", target_bir_lowering=False, debug=False,
                   num_devices=NCORES, num_swdge_queues=NQ)

    # ---- dram tensors ----
    din = {}
    def inp(name, shape, dt):
        din[name] = nc.dram_tensor(name, shape, dt, kind="ExternalInput")
        return din[name]

    xT_d = inp("xT", [512, NODE_PAD], bf16)
    idx_d = inp("idx", [128, COLS], i16)
    wext_d = [inp("wext1", [512, 136], bf16),
              inp("wext2", [128, 136], bf16),
              inp("wext3", [128, 66], bf16)]
    rep_d = []
    for li, (_, F, _, _) in enumerate(LAYER_DIMS):
        rep_d.append([inp(f"{nm}{li+1}r", [128, F], f32)
                      for nm in ("b", "g", "be")])
    bmat_d = inp("bmat", [128, NPAT * 128], bf16)
    emat_d = inp("emat", [128, NPAT * 128], bf16)
    ident_d = inp("ident", [128, 128], bf16)
    mask01_d = inp("mask01", [128, NBLK], f32)
    maskneg_d = inp("maskneg", [128, NBLK], f32)
    cw1_d = inp("cw1", [128, 128], f32)
    cb1_d = inp("cb1", [128, 1], f32)
    cw2_d = inp("cw2", [128, 2], f32)
    cb2_d = inp("cb2", [2, 1], f32)

    out_d = nc.dram_tensor("out", [1, 2], f32, kind="ExternalOutput")

    # table row = 256 B: h as fp8 (L1/L2, 128 vals) or bf16 (L3, 64 vals)
    # in bytes 0:128; als as bf16 in bytes 128:128+2H.
    ag_in = nc.dram_tensor("ag_in", [SR, 256], fp8)
    tfull = nc.dram_tensor("tfull", [NCORES * SR, 256], fp8,
                           addr_space="Shared")
    zin = nc.dram_tensor("zin", [128, 1], f32)
    zag = nc.dram_tensor("zag", [NCORES * 128, 1], f32, addr_space="Shared")

    rg = [list(range(NCORES))]
    qn = [0]  # rotating gather queue

    with tile.TileContext(nc) as tc:
        with tc.tile_pool(name="persist", bufs=1) as pp, \
             tc.tile_pool(name="work", bufs=3) as wp, \
             tc.tile_pool(name="gpool", bufs=3) as gp, \
             tc.tile_pool(name="pp_proj", bufs=2, space="PSUM") as ps_proj, \
             tc.tile_pool(name="pp_agg", bufs=2, space="PSUM") as ps_agg, \
             tc.tile_pool(name="pp_esc", bufs=2, space="PSUM") as ps_esc, \
             tc.tile_pool(name="pp_tr", bufs=2, space="PSUM") as ps_tr:

            # ---- load constants ----
            idx_sb = pp.tile([128, COLS], i16, tag="idx")
            nc.sync.dma_start(out=idx_sb[:, :], in_=idx_d[:, :])
            bmat_sb = pp.tile([128, NPAT * 128], bf16, tag="bmat")
            nc.sync.dma_start(out=bmat_sb[:, :], in_=bmat_d[:, :])
            emat_sb = pp.tile([128, NPAT * 128], bf16, tag="emat")
            nc.sync.dma_start(out=emat_sb[:, :], in_=emat_d[:, :])
            ident_sb = pp.tile([128, 128], bf16, tag="ident")
            nc.sync.dma_start(out=ident_sb[:, :], in_=ident_d[:, :])
            mask01_sb = pp.tile([128, NBLK], f32, tag="m01")
            nc.sync.dma_start(out=mask01_sb[:, :], in_=mask01_d[:, :])
            maskneg_sb = pp.tile([128, NBLK], f32, tag="mng")
            nc.sync.dma_start(out=maskneg_sb[:, :], in_=maskneg_d[:, :])
            cw1_sb = pp.tile([128, 128], f32, tag="cw1")
            nc.sync.dma_start(out=cw1_sb[:, :], in_=cw1_d[:, :])
            cb1_sb = pp.tile([128, 1], f32, tag="cb1")
            nc.sync.dma_start(out=cb1_sb[:, :], in_=cb1_d[:, :])
            cw2_sb = pp.tile([128, 2], f32, tag="cw2")
            nc.sync.dma_start(out=cw2_sb[:, :], in_=cw2_d[:, :])
            cb2_sb = pp.tile([2, 1], f32, tag="cb2")
            nc.sync.dma_start(out=cb2_sb[:, :], in_=cb2_d[:, :])
            wext_sb = []
            for li, (FIN, F, H, C) in enumerate(LAYER_DIMS):
                ncol = F + 2 * H
                t = pp.tile([128, (FIN // 128) * ncol], bf16, tag=f"wx{li}")
                for kc in range(FIN // 128):
                    nc.sync.dma_start(
                        out=t[:, kc * ncol:(kc + 1) * ncol],
                        in_=wext_d[li][kc * 128:(kc + 1) * 128, :])
                wext_sb.append(t)
            rep_sb = []
            for li, (_, F, _, _) in enumerate(LAYER_DIMS):
                row = []
                for di, d in enumerate(rep_d[li]):
                    t = pp.tile([128, F], f32, tag=f"rep{li}{di}")
                    nc.sync.dma_start(out=t[:, :], in_=d[:, :])
                    row.append(t)
                rep_sb.append(row)
            ident_f = pp.tile([128, 128], f32, tag="idf")
            nc.vector.tensor_copy(out=ident_f[:, :], in_=ident_sb[:, :])
            ones_f = pp.tile([128, 1], f32, tag="ones")
            nc.vector.memset(ones_f[:, :], 1.0)
            sent_sb = pp.tile([1, 256], fp8, tag="sent")
            nc.vector.memset(sent_sb[:, :], 0.0)

            # resident work tensors
            xT_sb = pp.tile([128, NODE_PAD], bf16, tag="xT")
            xn_sb = pp.tile([128, NBLK * 128], bf16, tag="xn")
            al_sb = pp.tile([128, NBLK * 8], bf16, tag="al")
            hloc_sb = pp.tile([128, NBLK * 128], bf16, tag="hloc")
            esc_all = pp.tile([128, 4 * TOTJ], f32, tag="escall")

            for _rep in range(repeat):
              nlayers = {"proj": 1, "proj_noag": 1, "l1": 1, "l2": 2,
                         "l2_noag": 2, "l2_nogather": 2,
                         "e_gather": 1, "e_esc": 1, "e_mult": 1,
                         "e_agg": 1}.get(stop, 3)
              do_edge = stop not in ("proj", "proj_noag")
              do_pool = stop is None
              elevel = {"e_gather": 1, "e_esc": 2, "e_mult": 3,
                        "e_agg": 4}.get(stop, 99)
              for li, (FIN, F, H, C) in enumerate(LAYER_DIMS[:nlayers]):
                  ncol = F + 2 * H
                  h_fp8 = li < 2          # L3 h fits bytes 0:128 as bf16
                  # ---------------- projection + table ----------------
                  sent_bf = sent_sb[:, :].bitcast(bf16)
                  nc.vector.memset(sent_bf[:, 64:64 + H], SENT_ALS)
                  for nb in range(NBLK):
                      psum_p = ps_proj.tile([128, 144], f32, tag="proj",
                                            space="PSUM")
                      nkc = FIN // 128
                      for kc in range(nkc):
                          if li == 0:
                              lhs = wp.tile([128, 128], bf16, tag="xTc")
                              nc.sync.dma_start(
                                  out=lhs[:, :],
                                  in_=xT_d[kc * 128:(kc + 1) * 128,
                                           nb * 128:(nb + 1) * 128])
                          else:
                              lhs = xT_sb[:, nb * 128:(nb + 1) * 128]
                          nc.tensor.matmul(
                              out=psum_p[:, :ncol],
                              lhsT=lhs[:, :] if li == 0 else lhs,
                              rhs=wext_sb[li][:, kc * ncol:(kc + 1) * ncol],
                              start=(kc == 0), stop=(kc == nkc - 1))
                      tab = wp.tile([128, 256], fp8, tag="tab")
                      tab_bf = tab[:, :].bitcast(bf16)
                      if h_fp8:
                          nc.vector.tensor_copy(out=tab[:, :F],
                                                in_=psum_p[:, :F])
                      else:
                          nc.vector.tensor_copy(out=tab_bf[:, :F],
                                                in_=psum_p[:, :F])
                      nc.vector.tensor_copy(out=tab_bf[:, 64:64 + H],
                                            in_=psum_p[:, F:F + H])
                      nc.vector.tensor_copy(
                          out=al_sb[:, nb * 8:nb * 8 + 2 * H],
                          in_=psum_p[:, F:F + 2 * H])
                      nc.vector.tensor_copy(
                          out=hloc_sb[:, nb * 128:nb * 128 + F],
                          in_=psum_p[:, :F])
                      nc.sync.dma_start(
                          out=ag_in[nb * 128:(nb + 1) * 128, :],
                          in_=tab[:, :])
                  nc.sync.dma_start(out=ag_in[SR - 1:SR, :],
                                    in_=sent_sb[:, :])
                  # esc precompute (PE churns through these during the AG)
                  if do_edge and elevel >= 2:
                      for nb in range(NBLK):
                          bl = p.blocks[nb]
                          J = bl["J"]
                          pe_esc = ps_esc.tile([128, 4 * JMAX], f32,
                                               tag="esc", space="PSUM")
                          for sbi, (side, k, j, si) in enumerate(
                                  bl["sblocks"]):
                              pid = p.pat_ids[(k, j, si)]
                              nc.tensor.matmul(
                                  out=pe_esc[:, sbi * H:(sbi + 1) * H],
                                  lhsT=emat_sb[:, pid * 128:(pid + 1) * 128],
                                  rhs=al_sb[:, nb * 8 + H:nb * 8 + 2 * H],
                                  start=True, stop=True)
                          nc.vector.tensor_copy(
                              out=esc_all[:, H * cumJ[nb]:
                                          H * cumJ[nb] + H * J],
                              in_=pe_esc[:, :H * J])
                  if not (stop == "proj_noag"
                          or (stop == "l2_noag" and li == 1)):
                      nc.gpsimd.collective_compute(
                          "AllGather", mybir.AluOpType.bypass,
                          replica_groups=rg,
                          ins=[ag_in.ap().opt()], outs=[tfull.ap().opt()])

                  # ---------------- edge phase (software-pipelined) -------
                  if not do_edge:
                      break
                  tfA = tfull[0:HALF, :]
                  tfB = tfull[HALF:2 * HALF, :]
                  GJ = F + H
                  Gs, exs, aggs = {}, {}, {}

                  def gather_stage(nb):
                      bl = p.blocks[nb]
                      J = bl["J"]
                      G = gp.tile([128, JMAX * 256], fp8, tag="G")
                      Gs[nb] = G
                      for (i0, ncall, side, col0) in bl["calls"]:
                          nc.gpsimd.dma_gather(
                              out_ap=G[:, i0 * 256:(i0 + ncall) * 256]
                              .rearrange("p (c e) -> p c e", e=256),
                              in_ap=tfA if side == "A" else tfB,
                              idxs_ap=idx_sb[:, col0:col0 + ncall * 8],
                              num_idxs=ncall * 128,
                              num_idxs_reg=ncall * 128,
                              elem_size=256, queue_num=qn[0] % NQ,
                              single_packet=SINGLE_PACKET)
                          qn[0] += 1
                      if elevel < 2:
                          return
                      # bridge copy in the writers' dtype (fp8): the dep
                      # tracker intersects ranges in element units, so a
                      # direct bf16 bitcast read of the multi-call gather
                      # output loses all but the first call's dependency.
                      als_t = wp.tile([128, 8 * JMAX], fp8, tag="alst")
                      nc.vector.tensor_copy(
                          out=als_t[:, :8 * J],
                          in_=G[:, :J * 256].rearrange(
                              "p (c e) -> p c e", e=256)[:, :, 128:136])
                      als_bf = als_t[:, :8 * J].bitcast(bf16).rearrange(
                          "p (c e) -> p c e", e=4)
                      e_f = wp.tile([128, 4 * JMAX], f32, tag="ef")
                      nc.vector.tensor_add(
                          out=e_f[:, :H * J].rearrange(
                              "p (c h) -> p c h", h=H),
                          in0=als_bf[:, :, 0:H],
                          in1=esc_all[:, H * cumJ[nb]:H * cumJ[nb] + H * J]
                          .rearrange("p (c h) -> p c h", h=H))
                      lr = wp.tile([128, 4 * JMAX], f32, tag="lr")
                      nc.vector.tensor_scalar(
                          out=lr[:, :H * J], in0=e_f[:, :H * J],
                          scalar1=0.2, scalar2=None,
                          op0=mybir.AluOpType.mult)
                      nc.vector.tensor_max(
                          out=e_f[:, :H * J], in0=e_f[:, :H * J],
                          in1=lr[:, :H * J])
                      ex = wp.tile([128, 4 * JMAX], bf16, tag="ex")
                      nc.scalar.activation(
                          out=ex[:, :H * J], in_=e_f[:, :H * J],
                          func=mybir.ActivationFunctionType.Exp)
                      exs[nb] = ex

                  def mult_stage(nb):
                      bl = p.blocks[nb]
                      J = bl["J"]
                      G, ex = Gs.pop(nb), exs.pop(nb)
                      Gv = G[:, :J * 256].rearrange("p (c e) -> p c e", e=256)
                      exv = ex[:, :H * J].rearrange("p (c h) -> p c h", h=H)
                      Gm = gp.tile([128, JMAX * 132], bf16, tag="Gm")
                      Gmv = Gm[:, :J * GJ].rearrange("p (c e) -> p c e", e=GJ)
                      if h_fp8:
                          h_src = Gv[:, :, 0:F]
                      else:
                          # same-dtype bridge for the bf16 h bitcast (L3)
                          ht = wp.tile([128, 128 * JMAX], fp8, tag="ht")
                          nc.vector.tensor_copy(out=ht[:, :J * 128],
                                                in_=Gv[:, :, 0:128])
                          h_src = ht[:, :J * 128].bitcast(bf16).rearrange(
                              "p (c e) -> p c e", e=64)[:, :, 0:F]
                      nc.vector.tensor_tensor(
                          out=Gmv[:, :, 0:F].rearrange(
                              "p c (h w) -> p c h w", h=H),
                          in0=h_src.rearrange("p c (h w) -> p c h w", h=H),
                          in1=exv.unsqueeze(3).to_broadcast([128, J, H, C]),
                          op=mybir.AluOpType.mult)
                      nc.vector.tensor_copy(out=Gmv[:, :, F:F + H], in_=exv)
                      if elevel < 4:
                          return
                      agg = ps_agg.tile([128, 144], f32, tag="agg",
                                        space="PSUM")
                      aggs[nb] = agg
                      for sbi, (side, k, j, si) in enumerate(bl["sblocks"]):
                          pid = p.pat_ids[(k, j, si)]
                          nc.tensor.matmul(
                              out=agg[:, :F + H],
                              lhsT=bmat_sb[:, pid * 128:(pid + 1) * 128],
                              rhs=Gm[:, sbi * GJ:sbi * GJ + F + H],
                              start=(sbi == 0), stop=(sbi == J - 1),
                              skip_group_check=True)

                  def post_stage(nb):
                      agg = aggs.pop(nb)
                      es = wp.tile([128, 4], f32, tag="es")
                      nc.vector.tensor_add(
                          out=es[:, :H], in0=al_sb[:, nb * 8:nb * 8 + H],
                          in1=al_sb[:, nb * 8 + H:nb * 8 + 2 * H])
                      lr2 = wp.tile([128, 4], f32, tag="lr2")
                      nc.vector.tensor_scalar(
                          out=lr2[:, :H], in0=es[:, :H], scalar1=0.2,
                          scalar2=None, op0=mybir.AluOpType.mult)
                      nc.vector.tensor_max(out=es[:, :H], in0=es[:, :H],
                                           in1=lr2[:, :H])
                      nc.scalar.activation(
                          out=es[:, :H], in_=es[:, :H],
                          func=mybir.ActivationFunctionType.Exp)
                      num = wp.tile([128, 128], f32, tag="num")
                      nc.vector.tensor_tensor(
                          out=num[:, :F].rearrange("p (h w) -> p h w", h=H),
                          in0=hloc_sb[:, nb * 128:nb * 128 + F].rearrange(
                              "p (h w) -> p h w", h=H),
                          in1=es[:, :H].unsqueeze(2).to_broadcast([128, H, C]),
                          op=mybir.AluOpType.mult)
                      nc.vector.tensor_add(out=num[:, :F], in0=num[:, :F],
                                           in1=agg[:, 0:F])
                      den = wp.tile([128, 4], f32, tag="den")
                      nc.vector.tensor_add(out=den[:, :H], in0=agg[:, F:F + H],
                                           in1=es[:, :H])
                      nc.vector.tensor_scalar_add(out=den[:, :H],
                                                  in0=den[:, :H],
                                                  scalar1=1e-16)
                      rec = wp.tile([128, 4], f32, tag="rec")
                      nc.vector.reciprocal(out=rec[:, :H], in_=den[:, :H])
                      xw = wp.tile([128, 128], f32, tag="xw")
                      nc.vector.tensor_tensor(
                          out=xw[:, :F].rearrange("p (h w) -> p h w", h=H),
                          in0=num[:, :F].rearrange("p (h w) -> p h w", h=H),
                          in1=rec[:, :H].unsqueeze(2).to_broadcast([128, H, C]),
                          op=mybir.AluOpType.mult)
                      nc.vector.tensor_add(out=xw[:, :F], in0=xw[:, :F],
                                           in1=rep_sb[li][0][:, :])
                      # layernorm
                      trash = wp.tile([128, 128], f32, tag="trash")
                      mu = wp.tile([128, 1], f32, tag="mu")
                      nc.scalar.activation(
                          out=trash[:, :F], in_=xw[:, :F],
                          func=mybir.ActivationFunctionType.Identity,
                          scale=1.0 / F, accum_out=mu[:, :])
                      cen = wp.tile([128, 128], f32, tag="cen")
                      nc.vector.tensor_scalar(
                          out=cen[:, :F], in0=xw[:, :F], scalar1=mu[:, :],
                          scalar2=None, op0=mybir.AluOpType.subtract)
                      sq = wp.tile([128, 1], f32, tag="sq")
                      nc.scalar.activation(
                          out=trash[:, :F], in_=cen[:, :F],
                          func=mybir.ActivationFunctionType.Square,
                          accum_out=sq[:, :])
                      rstd = wp.tile([128, 1], f32, tag="rstd")
                      nc.vector.tensor_scalar(
                          out=rstd[:, :], in0=sq[:, :], scalar1=1.0 / F,
                          scalar2=1e-5, op0=mybir.AluOpType.mult,
                          op1=mybir.AluOpType.add)
                      nc.scalar.activation(
                          out=rstd[:, :], in_=rstd[:, :],
                          func=mybir.ActivationFunctionType.Sqrt)
                      nc.vector.reciprocal(out=rstd[:, :], in_=rstd[:, :])
                      nc.vector.tensor_scalar(
                          out=cen[:, :F], in0=cen[:, :F], scalar1=rstd[:, :],
                          scalar2=None, op0=mybir.AluOpType.mult)
                      nc.vector.tensor_mul(out=cen[:, :F], in0=cen[:, :F],
                                           in1=rep_sb[li][1][:, :])
                      nc.vector.tensor_add(out=cen[:, :F], in0=cen[:, :F],
                                           in1=rep_sb[li][2][:, :])
                      # ELU
                      t3 = wp.tile([128, 128], f32, tag="t3")
                      nc.vector.tensor_scalar(
                          out=t3[:, :F], in0=cen[:, :F], scalar1=0.0,
                          scalar2=None, op0=mybir.AluOpType.min)
                      nc.scalar.activation(
                          out=t3[:, :F], in_=t3[:, :F],
                          func=mybir.ActivationFunctionType.Exp)
                      nc.vector.tensor_scalar_add(out=t3[:, :F],
                                                  in0=t3[:, :F], scalar1=-1.0)
                      nc.vector.tensor_tensor(
                          out=xn_sb[:, nb * F:(nb + 1) * F],
                          in0=cen[:, :F], in1=t3[:, :F],
                          op=mybir.AluOpType.max)

                  for it in range(NBLK + 2):
                      if it < NBLK:
                          gather_stage(it)
                      if elevel >= 3 and 0 <= it - 1 < NBLK:
                          mult_stage(it - 1)
                      if elevel >= 5 and 0 <= it - 2 < NBLK:
                          post_stage(it - 2)
                  # transpose xn -> xT for next layer
                  if li < nlayers - 1 and elevel >= 5:
                      for nb in range(NBLK):
                          pt = ps_tr.tile([128, 128], bf16, tag="tr",
                                          space="PSUM")
                          nc.tensor.transpose(
                              out=pt[:, :],
                              in_=xn_sb[:, nb * 128:(nb + 1) * 128],
                              identity=ident_sb[:, :])
                          nc.vector.tensor_copy(
                              out=xT_sb[:, nb * 128:(nb + 1) * 128],
                              in_=pt[:, :])

              # ---------------- pooling + classifier ----------------
              if not do_pool:
                  o0 = wp.tile([1, 2], f32, tag="o0")
                  nc.vector.memset(o0[:, :], 0.0)
                  nc.sync.dma_start(out=out_d[:, :], in_=o0[:, :])
              else:
                F3 = 64
                sumacc = pp.tile([128, F3], f32, tag="sumacc")
                nc.vector.memset(sumacc[:, :], 0.0)
                maxacc = pp.tile([128, F3], f32, tag="maxacc")
                nc.vector.memset(maxacc[:, :], -1e9)
                for nb in range(NBLK):
                    blk = xn_sb[:, nb * F3:(nb + 1) * F3]
                    hm = wp.tile([128, F3], f32, tag="hm")
                    nc.vector.tensor_scalar(
                        out=hm[:, :], in0=blk, scalar1=mask01_sb[:, nb:nb + 1],
                        scalar2=None, op0=mybir.AluOpType.mult)
                    nc.vector.tensor_add(out=sumacc[:, :], in0=sumacc[:, :],
                                         in1=hm[:, :])
                    nc.vector.tensor_scalar(
                        out=hm[:, :], in0=blk, scalar1=maskneg_sb[:, nb:nb + 1],
                        scalar2=None, op0=mybir.AluOpType.add)
                    nc.vector.tensor_max(out=maxacc[:, :], in0=maxacc[:, :],
                                         in1=hm[:, :])
                sps = ps_tr.tile([64, 1], f32, tag="tr", space="PSUM")
                nc.tensor.matmul(out=sps[:, :], lhsT=sumacc[:, :],
                                 rhs=ones_f[:, :], start=True, stop=True)
                s1 = wp.tile([64, 1], f32, tag="s1")
                nc.vector.tensor_copy(out=s1[:, :], in_=sps[:, :])
                mps = ps_tr.tile([64, 128], f32, tag="tr", space="PSUM")
                nc.tensor.matmul(out=mps[:, :], lhsT=maxacc[:, :],
                                 rhs=ident_f[:, :], start=True, stop=True,
                                 is_transpose=True)
                m1 = wp.tile([64, 1], f32, tag="m1")
                nc.vector.tensor_reduce(out=m1[:, :], in_=mps[:, :],
                                        axis=mybir.AxisListType.X,
                                        op=mybir.AluOpType.max)
                nc.sync.dma_start(out=zin[0:64, :], in_=s1[:, :])
                nc.sync.dma_start(out=zin[64:128, :], in_=m1[:, :])
                nc.gpsimd.collective_compute(
                    "AllGather", mybir.AluOpType.bypass, replica_groups=rg,
                    ins=[zin.ap().opt()], outs=[zag.ap().opt()])
                zs = wp.tile([128, 8], f32, tag="zs")
                for c in range(NCORES):
                    nc.sync.dma_start(out=zs[:, c:c + 1],
                                      in_=zag[c * 128:(c + 1) * 128, :])
                z2 = wp.tile([128, 1], f32, tag="z2")
                nc.vector.tensor_reduce(out=z2[0:64, :], in_=zs[0:64, :],
                                        axis=mybir.AxisListType.X,
                                        op=mybir.AluOpType.add)
                nc.vector.tensor_reduce(out=z2[64:128, :], in_=zs[64:128, :],
                                        axis=mybir.AxisListType.X,
                                        op=mybir.AluOpType.max)
                f1 = ps_tr.tile([128, 1], f32, tag="tr", space="PSUM")
                nc.tensor.matmul(out=f1[:, :], lhsT=cw1_sb[:, :], rhs=z2[:, :],
                                 start=True, stop=True)
                r1 = wp.tile([128, 1], f32, tag="r1")
                nc.scalar.activation(out=r1[:, :], in_=f1[:, :],
                                     func=mybir.ActivationFunctionType.Relu,
                                     bias=cb1_sb[:, :])
                f2 = ps_tr.tile([2, 1], f32, tag="tr", space="PSUM")
                nc.tensor.matmul(out=f2[:, :], lhsT=cw2_sb[:, :], rhs=r1[:, :],
                                 start=True, stop=True)
                o = wp.tile([2, 1], f32, tag="o")
                nc.vector.tensor_add(out=o[:, :], in0=f2[:, :], in1=cb2_sb[:, :])
                nc.sync.dma_start(out=out_d[:, :], in_=o[:, :])
    nc.compile()
    return nc


def build_in_maps(p, inputs):
    """Per-core input dicts."""
    x = np.asarray(inputs["x"], np.float32)
    FIN = x.shape[1]
    in_maps = []
    gi = lambda k: np.asarray(inputs[k], np.float32)

    wext, reps = [], []
    for li, (fin, F, H, C) in enumerate(LAYER_DIMS):
        W = gi(f"W{li+1}")
        a_s, a_d = gi(f"as{li+1}"), gi(f"ad{li+1}")
        as_bd = np.zeros((H * C, H), np.float32)
        ad_bd = np.zeros((H * C, H), np.float32)
        for h in range(H):
            as_bd[h * C:(h + 1) * C, h] = a_s[h]
            ad_bd[h * C:(h + 1) * C, h] = a_d[h]
        wext.append(np.concatenate(
            [W, W @ as_bd, W @ ad_bd], axis=1).astype(bf16_np))
        reps.append([np.tile(gi(f"{nm}{li+1}")[None, :], (128, 1))
                     .astype(np.float32) for nm in ("b", "g", "be")])

    ident = np.eye(128, dtype=bf16_np)
    cw1 = gi("cW1").copy()
    cw1[:64, :] /= p.N          # fold mean divisor into cW1 rows
    cb1 = gi("cb1")[:, None].astype(np.float32)
    cw2 = gi("cW2").astype(np.float32)
    cb2 = gi("cb2")[:, None].astype(np.float32)

    for c in range(NCORES):
        m = {}
        xp = np.zeros((p.NODE_PAD, FIN), np.float32)
        valid = p.perm[c] >= 0
        xp[valid] = x[p.perm[c][valid]]
        m["xT"] = np.ascontiguousarray(xp.T).astype(bf16_np)
        m["idx"] = p.idx_stream[c]
        for li in range(3):
            m[f"wext{li+1}"] = wext[li]
            for di, nm in enumerate(("b", "g", "be")):
                m[f"{nm}{li+1}r"] = reps[li][di]
        m["bmat"] = p.bmat
        m["emat"] = p.emat
        m["ident"] = ident
        mk = valid.astype(np.float32)
        m["mask01"] = mk.reshape(p.NBLK, 128).T.copy()
        m["maskneg"] = ((1.0 - mk) * -1e9).reshape(p.NBLK, 128).T.copy()
        m["cw1"] = cw1.astype(np.float32)
        m["cb1"] = cb1
        m["cw2"] = cw2
        m["cb2"] = cb2
        in_maps.append(m)
    return in_maps


_CACHE = {}


def kernel(**inputs):
    from concourse.bass_utils import run_bass_kernel_spmd
    os.environ.setdefault("NEURON_RT_RESET_CORES", "1")
    key = "k"
    if key not in _CACHE:
        p = build_plan(np.asarray(inputs["edge_index"]),
                       int(np.asarray(inputs["x"]).shape[0]))
        nc = build_bass(p)
        _CACHE[key] = (p, nc)
    p, nc = _CACHE[key]
    in_maps = build_in_maps(p, inputs)
    res = run_bass_kernel_spmd(nc, in_maps, core_ids=list(range(NCORES)))
    return res.results[0]["out"].astype(np.float32)

